# revision 13
# baseline (speedup 1.0000x reference)
"""Trainium2 Bass kernel for nn_BraidCrossing (B=8, T=2048, D=2048, NG=3).

Math notes
----------
reference computes:
    pair  = [x_t, x_{t+1}]                       (B, T-1, 2D)
    h     = gelu(pair @ W1.T + b1)
    logit = h @ W2.T + b2                        (B, T-1, 2*NG)
    scale = mean(softmax(logit, -1), -1)         == 1/(2*NG) EXACTLY (mean of a
                                                 softmax over the same axis)
    P     = x @ Wp.T + bp
    tmp_t = LN(x_t + P_{t-1} * scale)   t>=1 ;  tmp_0 = x_0
    out_t = LN(tmp_t + P_{t+1} * scale) t<=T-2; out_{T-1} = tmp_{T-1}

Because scale is a constant (1/(2*NG); setup has bp=0, gamma=1, beta=0), the
entire W1/W2/gelu branch is dead code.  The device kernel computes
Q = (x @ Wp.T) * scale, then the two chained layernorms.

Precision: the GEMM runs in fp8 e4m3 (inputs pre-scaled into the normal
range, DoubleRow perf mode: K=256 per matmul, fp32 PSUM accumulation); the
LN chain runs in bf16 with fp32 statistics and fp32 output.  Measured
end-to-end max rel err ~1.1e-2 vs the fp32 reference (gate is 2e-2).

Sharding: data-parallel over batch, one batch per NeuronCore (8 cores).
"""
import numpy as np
import ml_dtypes

import concourse.bass as bass
from concourse import bacc
import concourse.mybir as mybir
import concourse.tile as tile
from concourse.bass_utils import run_bass_kernel_spmd

FP32 = mybir.dt.float32
BF16 = mybir.dt.bfloat16
F8 = mybir.dt.float8e4
AF = mybir.ActivationFunctionType
ALU = mybir.AluOpType
DR = mybir.MatmulPerfMode.DoubleRow

B, T, D = 8, 2048, 2048
P = 128                # partitions
NT = T // P            # 16 t-tiles
NK = D // P            # 16 contraction k-tiles
NKP = NK // 2          # 8 k-pairs (DoubleRow: 256 contraction per matmul)
NE = D // 512          # 4 psum-bank chunks along e
EPS = 1e-5
N_CORES = 8

SX = 16.0              # fp8 pre-scale for x
SW = 1024.0            # fp8 pre-scale for Wp.T
F8NP = ml_dtypes.float8_e4m3
BF16NP = ml_dtypes.bfloat16

_cache = {}


def _build(scale: float, T: int = T):
    NT = T // P
    # combined scale folded into the PSUM->SBUF copy:
    # PSUM = (x*SX) @ (WpT*SW)  ->  q = PSUM * (scale / (SX*SW))
    qscale = float(scale) / (SX * SW)

    nc = bacc.Bacc("TRN2", target_bir_lowering=False, debug=False)
    xb_d = nc.declare_dram_parameter("xb", [T, D], BF16, isOutput=False)
    # host-tiled transpose: xTt[i, p, k, tt] = x[i*128+tt, k*128+p] (fp8),
    # so lhsT slice [:, 2kp:2kp+2, :] is the DoubleRow stationary operand
    xTt_d = nc.declare_dram_parameter("xTt", [NT, P, NK, P], F8, isOutput=False)
    # w8[kp, p, s, e] = WpT[(2kp+s)*128+p, e] * SW (fp8)
    w8_d = nc.declare_dram_parameter("w8", [NKP, P, 2, D], F8, isOutput=False)
    out_d = nc.declare_dram_parameter("out", [T, D], FP32, isOutput=True)

    xb_ap = xb_d.ap()
    out_ap = out_d.ap()
    xTt_ap = xTt_d.ap()

    with tile.TileContext(nc) as tc:
        with tc.tile_pool(name="wp", bufs=1) as wp_pool, \
             tc.tile_pool(name="xt", bufs=3) as xt_pool, \
             tc.tile_pool(name="q", bufs=5) as q_pool, \
             tc.tile_pool(name="v", bufs=6) as v_pool, \
             tc.tile_pool(name="o", bufs=3) as o_pool, \
             tc.tile_pool(name="stat", bufs=8) as stat_pool, \
             tc.tile_pool(name="ps", bufs=8, space="PSUM") as ps_pool:

            eps_t = stat_pool.tile([P, 1], FP32, tag="eps", bufs=1)
            nc.vector.memset(eps_t, EPS)

            # resident fp8 weights: 8 k-pair tiles of (128, 2, 2048), split
            # across the two HWDGE queues so the 4 MiB stream drains fast
            wp = []
            for kp in range(NKP):
                w = wp_pool.tile([P, 2, D], F8, tag=f"wp{kp}", bufs=1)
                eng = nc.sync if kp % 2 == 0 else nc.scalar
                eng.dma_start(out=w, in_=w8_d.ap()[kp])
                wp.append(w)

            # prefetch the first two t-tiles' lhsT after the weights on the
            # sync queue so the PE burst starts dense once weights landed
            xt_pre = {}
            for i in (NT - 1, NT - 2):
                xt_i = xt_pool.tile([P, NK, P], F8, tag="xt")
                nc.sync.dma_start(out=xt_i, in_=xTt_ap[i])
                xt_pre[i] = xt_i

            def ln_stats(v, nparts, nchunk=1, want_nmr=True):
                """rs, nmr tiles: per-row 1/sigma and -mu/sigma of v."""
                stats = stat_pool.tile([P, NE, 6], FP32, tag="stats")
                mv = stat_pool.tile([P, 2], FP32, tag="mv")
                for c in range(NE):
                    nc.vector.bn_stats(out=stats[:nparts, c, :],
                                       in_=v[:nparts, c * 512:(c + 1) * 512])
                nc.vector.bn_aggr(out=mv[:nparts], in_=stats[:nparts])
                rs = stat_pool.tile([P, 1], FP32, tag="rs")
                nc.scalar.activation(out=rs[:nparts], in_=mv[:nparts, 1:2],
                                     func=AF.Sqrt, bias=eps_t[:nparts], scale=1.0)
                nc.vector.reciprocal(out=rs[:nparts], in_=rs[:nparts])
                if not want_nmr:
                    return mv, rs, None
                nmr = stat_pool.tile([P, 1], FP32, tag="nmr")
                nc.vector.scalar_tensor_tensor(out=nmr[:nparts],
                                               in0=mv[:nparts, 0:1], scalar=-1.0,
                                               in1=rs[:nparts],
                                               op0=ALU.mult, op1=ALU.mult)
                return mv, rs, nmr

            def apply_act(dst, src, nparts, rs, nmr, chunks=1):
                """dst = (src - mu) / sigma via ACT: Identity(src*rs + nmr)."""
                cw = D // chunks
                for c in range(chunks):
                    nc.scalar.activation(out=dst[:nparts, c * cw:(c + 1) * cw],
                                         in_=src[:nparts, c * cw:(c + 1) * cw],
                                         func=AF.Identity, bias=nmr[:nparts],
                                         scale=rs[:nparts])

            def apply_dve(dst, src, nparts, rs, mv, chunks=1):
                cw = D // chunks
                for c in range(chunks):
                    nc.vector.tensor_scalar(
                        out=dst[:nparts, c * cw:(c + 1) * cw],
                        in0=src[:nparts, c * cw:(c + 1) * cw],
                        scalar1=mv[:nparts, 0:1], scalar2=rs[:nparts],
                        op0=ALU.subtract, op1=ALU.mult)

            # Software-pipelined schedule: the MM+evacuate "front" for
            # tile i is emitted DELAY iterations ahead of tile i's LN chain,
            # so the ACT-queue FIFO (strict, in-order) never makes the next
            # tile's PSUM evacuation wait behind this tile's LN applies.
            # Tiles run in REVERSE order (15..0): tile i's v2 tail rows need
            # Q rows from tile i+1, produced by an earlier front.
            DELAY = 2
            st = {}

            def emit_front(i):
                ns = P if i < NT - 1 else P - 1
                xt_i = xt_pre.pop(i)
                qp = ps_pool.tile([P, D], FP32, tag="qps", bufs=2)
                for kp in range(NKP):
                    lhsT = xt_i[:, 2 * kp:2 * kp + 2, :]
                    for n in range(NE):
                        nc.tensor.matmul(qp[:, n * 512:(n + 1) * 512],
                                         lhsT,
                                         wp[kp][:, :, n * 512:(n + 1) * 512],
                                         start=(kp == 0), stop=(kp == NKP - 1),
                                         perf_mode=DR)
                if i - 1 >= 0 and (i - 1) not in xt_pre:
                    xt_n = xt_pool.tile([P, NK, P], F8, tag="xt")
                    nc.sync.dma_start(out=xt_n, in_=xTt_ap[i - 1])
                    xt_pre[i - 1] = xt_n

                tail_tile = i <= 1
                # PSUM evacuation (bf16 q tile, scaled) -- keeps PSUM free
                # for the next front regardless of LN-chain progress
                q_i = q_pool.tile([P, D], BF16, tag="q")
                nc.scalar.activation(out=q_i[:], in_=qp[:], func=AF.Copy,
                                     scale=qscale)
                # x rows for v1 land early too (independent sync-ring load)
                v1 = v_pool.tile([P, D], BF16, tag="v")
                nc.sync.dma_start(out=v1[:ns, :],
                                  in_=xb_ap[i * P + 1: i * P + 1 + ns, :])
                st[i] = dict(qp=qp, q=q_i, v1=v1, ns=ns)

            def emit_chain(i):
                s = st.pop(i)
                qp, q_i, v1, ns = s["qp"], s["q"], s["v1"], s["ns"]
                q_next = st[i + 1]["q"] if i + 1 in st else q_prev.get(i + 1)
                tail_tile = i <= 1
                nchunk = 2 if tail_tile else 1

                if tail_tile:
                    # read Q straight from PSUM: skips the q-copy dependency
                    # on the drained tail chains
                    for c in range(NE):
                        sl = slice(c * 512, (c + 1) * 512)
                        nc.vector.scalar_tensor_tensor(
                            out=v1[:ns, sl], in0=qp[:ns, sl], scalar=qscale,
                            in1=v1[:ns, sl], op0=ALU.mult, op1=ALU.add)
                else:
                    nc.vector.tensor_add(out=v1[:ns, :], in0=v1[:ns, :],
                                         in1=q_i[:ns, :])

                mv1, rs1, nmr1 = ln_stats(v1, ns, nchunk)
                v2 = v_pool.tile([P, D], BF16, tag="v")
                apply_act(v2, v1, ns, rs1, nmr1, chunks=nchunk)

                if i == NT - 1:
                    # out[T-1] = tmp row 126 (SWDGE DMA casts bf16 -> fp32)
                    nc.gpsimd.dma_start(out=out_ap[T - 1:T, :],
                                        in_=v2[126:127, :])

                # v2 += Q[t+1 rows]: partition shift fused into SWDGE accum
                no2 = P if i < NT - 1 else P - 2
                nc.gpsimd.dma_start(out=v2[0:126, :], in_=q_i[2:P, :],
                                    accum_op=ALU.add)
                if i < NT - 1:
                    nc.gpsimd.dma_start(out=v2[126:128, :], in_=q_next[0:2, :],
                                        accum_op=ALU.add)

                mv2, rs2, nmr2 = ln_stats(v2, no2, nchunk, want_nmr=True)
                out_t = o_pool.tile([P, D], FP32, tag="o")
                apply_act(out_t, v2, no2, rs2, nmr2, chunks=nchunk)
                nc.scalar.dma_start(out=out_ap[i * P + 1: i * P + 1 + no2, :],
                                    in_=out_t[:no2, :])

                if i == 0:
                    # out[0] = LN(x[0] + Q[1]): 1-row boundary
                    qrow = v_pool.tile([1, D], BF16, tag="qrow", bufs=1)
                    nc.sync.dma_start(out=qrow, in_=q_i[1:2, :])
                    v0 = v_pool.tile([1, D], BF16, tag="v0", bufs=1)
                    nc.sync.dma_start(out=v0[0:1, :], in_=xb_ap[0:1, :])
                    nc.vector.tensor_add(out=v0[0:1, :], in0=v0[0:1, :],
                                         in1=qrow[0:1, :])
                    mv0, rs0, _ = ln_stats(v0, 1, want_nmr=False)
                    o0 = o_pool.tile([1, D], FP32, tag="o0", bufs=1)
                    apply_dve(o0, v0, 1, rs0, mv0)
                    nc.sync.dma_start(out=out_ap[0:1, :], in_=o0[0:1, :])

                q_prev[i] = q_i

            q_prev = {}
            for i in reversed(range(NT)):
                emit_front(i)
                j = i + DELAY
                if j <= NT - 1:
                    emit_chain(j)
            for j in range(DELAY - 1, -1, -1):
                emit_chain(j)

    nc.compile()
    return nc


def _get_program(scale: float):
    key = round(float(scale), 9)
    if key not in _cache:
        _cache[key] = _build(float(scale))
    return _cache[key]


def _identity_ln_params(bp, gamma, beta):
    return (not np.any(bp)) and (not np.any(beta)) and np.all(gamma == 1.0)


def _reference_numpy(x, W1, b1, W2, b2, Wp, bp, gamma, beta):
    """Exact numpy port of the jax reference (emergency fallback only)."""
    import math

    def ln(v):
        mu = v.mean(-1, keepdims=True)
        var = ((v - mu) ** 2).mean(-1, keepdims=True)
        return (v - mu) / np.sqrt(var + EPS) * gamma + beta

    erf = np.vectorize(math.erf)
    x64 = x.astype(np.float32)
    pair = np.concatenate([x64[:, :-1], x64[:, 1:]], axis=-1)
    h0 = pair @ W1.T + b1
    h = 0.5 * h0 * (1.0 + erf(h0 / np.sqrt(2.0)))
    logits = h @ W2.T + b2
    e = np.exp(logits - logits.max(-1, keepdims=True))
    sm = e / e.sum(-1, keepdims=True)
    scale = sm.mean(-1, keepdims=True)
    Pm = x64 @ Wp.T + bp
    m = Pm[:, 1:] * scale
    mp = Pm[:, :-1] * scale
    tmp = np.concatenate([x64[:, :1], ln(x64[:, 1:] + mp)], axis=1)
    out = np.concatenate([ln(tmp[:, :-1] + m), tmp[:, -1:]], axis=1)
    return out.astype(np.float32)


def run_device(x, wT, scale, trace=False):
    """x: (B,T,D) fp32, wT: (D,D) fp32 (= Wp.T contiguous)."""
    nc = _get_program(scale)
    x8 = np.clip(x * SX, -240.0, 240.0).astype(F8NP)         # (B,T,D) fp8
    w8 = np.ascontiguousarray(
        np.clip(wT * SW, -240.0, 240.0).astype(F8NP)
        .reshape(NKP, 2, P, D).transpose(0, 2, 1, 3))        # (8,128,2,2048)
    in_maps = []
    for c in range(N_CORES):
        xb = np.ascontiguousarray(x[c].astype(BF16NP))
        # xTt[i, p, k, tt] = x8[i*128+tt, k*128+p]
        xTb = np.ascontiguousarray(
            x8[c].reshape(NT, P, NK, P).transpose(0, 3, 2, 1))
        in_maps.append({"xb": xb, "xTt": xTb, "w8": w8})
    res = run_bass_kernel_spmd(nc, in_maps, list(range(N_CORES)), trace=trace)
    out = np.stack([res.results[c]["out"] for c in range(N_CORES)], axis=0)
    return out, res


def kernel(x, W1, b1, W2, b2, Wp, bp, gamma, beta):
    x = np.asarray(x, dtype=np.float32)
    Wp = np.asarray(Wp, dtype=np.float32)
    bp = np.asarray(bp); gamma = np.asarray(gamma); beta = np.asarray(beta)
    b2 = np.asarray(b2)
    if x.shape != (B, T, D) or not _identity_ln_params(bp, gamma, beta):
        return _reference_numpy(np.asarray(x), np.asarray(W1), np.asarray(b1),
                                np.asarray(W2), b2, Wp, bp, gamma, beta)
    scale = 1.0 / float(b2.shape[0])
    wT = np.ascontiguousarray(Wp.T)
    out, _ = run_device(x, wT, scale, trace=False)
    return out


# revision 14
# speedup vs baseline: 1.3074x; 1.3074x over previous
"""Trainium2 Bass kernel for nn_BraidCrossing (B=8, T=2048, D=2048, NG=3).

Math notes
----------
reference computes:
    pair  = [x_t, x_{t+1}]                       (B, T-1, 2D)
    h     = gelu(pair @ W1.T + b1)
    logit = h @ W2.T + b2                        (B, T-1, 2*NG)
    scale = mean(softmax(logit, -1), -1)         == 1/(2*NG) EXACTLY (mean of a
                                                 softmax over the same axis)
    P     = x @ Wp.T + bp
    tmp_t = LN(x_t + P_{t-1} * scale)   t>=1 ;  tmp_0 = x_0
    out_t = LN(tmp_t + P_{t+1} * scale) t<=T-2; out_{T-1} = tmp_{T-1}

Because scale is a constant (1/(2*NG); setup has bp=0, gamma=1, beta=0), the
entire W1/W2/gelu branch is dead code.  The device kernel computes
Q = (x @ Wp.T) * scale, then the two chained layernorms.

Precision: the GEMM runs in fp8 e4m3 (inputs pre-scaled into the normal
range, DoubleRow perf mode: K=256 per matmul, fp32 PSUM accumulation); the
LN chain runs in bf16 with fp32 statistics and fp32 output.  Measured
end-to-end max rel err ~1.1e-2 vs the fp32 reference (gate is 2e-2).

Sharding: data-parallel over batch, one batch per NeuronCore (8 cores).
"""
import numpy as np
import ml_dtypes

import concourse.bass as bass
from concourse import bacc
import concourse.mybir as mybir
import concourse.tile as tile
from concourse.bass_utils import run_bass_kernel_spmd

FP32 = mybir.dt.float32
BF16 = mybir.dt.bfloat16
F8 = mybir.dt.float8e4
AF = mybir.ActivationFunctionType
ALU = mybir.AluOpType
DR = mybir.MatmulPerfMode.DoubleRow

B, T, D = 8, 2048, 2048
P = 128                # partitions
NT = T // P            # 16 t-tiles
NK = D // P            # 16 contraction k-tiles
NKP = NK // 2          # 8 k-pairs (DoubleRow: 256 contraction per matmul)
NE = D // 512          # 4 psum-bank chunks along e
EPS = 1e-5
N_CORES = 8

SX = 16.0              # fp8 pre-scale for x
SW = 1024.0            # fp8 pre-scale for Wp.T
F8NP = ml_dtypes.float8_e4m3
BF16NP = ml_dtypes.bfloat16

_cache = {}


def _build(scale: float, T: int = T):
    NT = T // P
    # combined scale folded into the PSUM->SBUF copy:
    # PSUM = (x*SX) @ (WpT*SW)  ->  q = PSUM * (scale / (SX*SW))
    qscale = float(scale) / (SX * SW)

    nc = bacc.Bacc("TRN2", target_bir_lowering=False, debug=False)
    xb_d = nc.declare_dram_parameter("xb", [T, D], BF16, isOutput=False)
    # host-tiled transpose: xTt[i, p, k, tt] = x[i*128+tt, k*128+p] (fp8),
    # so lhsT slice [:, 2kp:2kp+2, :] is the DoubleRow stationary operand
    xTt_d = nc.declare_dram_parameter("xTt", [NT, P, NK, P], F8, isOutput=False)
    # w8[kp, p, s, e] = WpT[(2kp+s)*128+p, e] * SW (fp8)
    w8_d = nc.declare_dram_parameter("w8", [NKP, P, 2, D], F8, isOutput=False)
    out_d = nc.declare_dram_parameter("out", [T, D], FP32, isOutput=True)

    xb_ap = xb_d.ap()
    out_ap = out_d.ap()
    xTt_ap = xTt_d.ap()

    with tile.TileContext(nc) as tc:
        with tc.tile_pool(name="wp", bufs=1) as wp_pool, \
             tc.tile_pool(name="xt", bufs=3) as xt_pool, \
             tc.tile_pool(name="q", bufs=5) as q_pool, \
             tc.tile_pool(name="v", bufs=6) as v_pool, \
             tc.tile_pool(name="o", bufs=3) as o_pool, \
             tc.tile_pool(name="stat", bufs=8) as stat_pool, \
             tc.tile_pool(name="ps", bufs=8, space="PSUM") as ps_pool:

            eps_t = stat_pool.tile([P, 1], FP32, tag="eps", bufs=1)
            nc.vector.memset(eps_t, EPS)

            # resident fp8 weights: 8 k-pair tiles of (128, 2, 2048), split
            # across the two HWDGE queues so the 4 MiB stream drains fast
            wp = []
            for kp in range(NKP):
                w = wp_pool.tile([P, 2, D], F8, tag=f"wp{kp}", bufs=1)
                eng = nc.sync if kp % 2 == 0 else nc.scalar
                eng.dma_start(out=w, in_=w8_d.ap()[kp])
                wp.append(w)

            # prefetch the first two t-tiles' lhsT after the weights on the
            # sync queue so the PE burst starts dense once weights landed
            xt_pre = {}
            for i in (NT - 1, NT - 2):
                xt_i = xt_pool.tile([P, NK, P], F8, tag="xt")
                nc.sync.dma_start(out=xt_i, in_=xTt_ap[i])
                xt_pre[i] = xt_i

            def ln_stats(v, nparts, nchunk=1, want_nmr=True):
                """rs, nmr tiles: per-row 1/sigma and -mu/sigma of v."""
                stats = stat_pool.tile([P, NE, 6], FP32, tag="stats")
                mv = stat_pool.tile([P, 2], FP32, tag="mv")
                for c in range(NE):
                    nc.vector.bn_stats(out=stats[:nparts, c, :],
                                       in_=v[:nparts, c * 512:(c + 1) * 512])
                nc.vector.bn_aggr(out=mv[:nparts], in_=stats[:nparts])
                rs = stat_pool.tile([P, 1], FP32, tag="rs")
                nc.scalar.activation(out=rs[:nparts], in_=mv[:nparts, 1:2],
                                     func=AF.Sqrt, bias=eps_t[:nparts], scale=1.0)
                nc.vector.reciprocal(out=rs[:nparts], in_=rs[:nparts])
                if not want_nmr:
                    return mv, rs, None
                nmr = stat_pool.tile([P, 1], FP32, tag="nmr")
                nc.vector.scalar_tensor_tensor(out=nmr[:nparts],
                                               in0=mv[:nparts, 0:1], scalar=-1.0,
                                               in1=rs[:nparts],
                                               op0=ALU.mult, op1=ALU.mult)
                return mv, rs, nmr

            def apply_act(dst, src, nparts, rs, nmr, chunks=1):
                """dst = (src - mu) / sigma via ACT: Identity(src*rs + nmr)."""
                cw = D // chunks
                for c in range(chunks):
                    nc.scalar.activation(out=dst[:nparts, c * cw:(c + 1) * cw],
                                         in_=src[:nparts, c * cw:(c + 1) * cw],
                                         func=AF.Identity, bias=nmr[:nparts],
                                         scale=rs[:nparts])

            def apply_dve(dst, src, nparts, rs, mv, chunks=1):
                cw = D // chunks
                for c in range(chunks):
                    nc.vector.tensor_scalar(
                        out=dst[:nparts, c * cw:(c + 1) * cw],
                        in0=src[:nparts, c * cw:(c + 1) * cw],
                        scalar1=mv[:nparts, 0:1], scalar2=rs[:nparts],
                        op0=ALU.subtract, op1=ALU.mult)

            # Software-pipelined schedule: the MM+evacuate "front" for
            # tile i is emitted DELAY iterations ahead of tile i's LN chain,
            # so the ACT-queue FIFO (strict, in-order) never makes the next
            # tile's PSUM evacuation wait behind this tile's LN applies.
            # Tiles run in REVERSE order (15..0): tile i's v2 tail rows need
            # Q rows from tile i+1, produced by an earlier front.
            DELAY = 2
            st = {}

            def emit_front(i):
                ns = P if i < NT - 1 else P - 1
                xt_i = xt_pre.pop(i)
                qp = ps_pool.tile([P, D], FP32, tag="qps", bufs=2)
                for kp in range(NKP):
                    lhsT = xt_i[:, 2 * kp:2 * kp + 2, :]
                    for n in range(NE):
                        nc.tensor.matmul(qp[:, n * 512:(n + 1) * 512],
                                         lhsT,
                                         wp[kp][:, :, n * 512:(n + 1) * 512],
                                         start=(kp == 0), stop=(kp == NKP - 1),
                                         perf_mode=DR)
                if i - 1 >= 0 and (i - 1) not in xt_pre:
                    xt_n = xt_pool.tile([P, NK, P], F8, tag="xt")
                    nc.sync.dma_start(out=xt_n, in_=xTt_ap[i - 1])
                    xt_pre[i - 1] = xt_n

                tail_tile = i <= 1
                # PSUM evacuation (bf16 q tile, scaled) -- keeps PSUM free
                # for the next front regardless of LN-chain progress
                q_i = q_pool.tile([P, D], BF16, tag="q")
                nc.scalar.activation(out=q_i[:], in_=qp[:], func=AF.Copy,
                                     scale=qscale)
                # x rows for v1 land early too (independent sync-ring load)
                v1 = v_pool.tile([P, D], BF16, tag="v")
                nc.sync.dma_start(out=v1[:ns, :],
                                  in_=xb_ap[i * P + 1: i * P + 1 + ns, :])
                # shifted Q rows for the second LN, built here (front phase)
                # from q_i and the previous front's q tile -- plain HWDGE
                # copies with ~2 tiles of slack before the chain reads them
                qs = v_pool.tile([P, D], BF16, tag="qs")
                nc.sync.dma_start(out=qs[0:126, :], in_=q_i[2:P, :])
                if i < NT - 1:
                    nc.sync.dma_start(out=qs[126:128, :],
                                      in_=st[i + 1]["q"][0:2, :])
                st[i] = dict(qp=qp, q=q_i, v1=v1, qs=qs, ns=ns)

            def emit_chain(i):
                s = st.pop(i)
                qp, q_i, v1, qs, ns = s["qp"], s["q"], s["v1"], s["qs"], s["ns"]
                tail_tile = i <= 1
                nchunk = 2 if tail_tile else 1

                if tail_tile:
                    # read Q straight from PSUM: skips the q-copy dependency
                    # on the drained tail chains
                    for c in range(NE):
                        sl = slice(c * 512, (c + 1) * 512)
                        nc.vector.scalar_tensor_tensor(
                            out=v1[:ns, sl], in0=qp[:ns, sl], scalar=qscale,
                            in1=v1[:ns, sl], op0=ALU.mult, op1=ALU.add)
                else:
                    nc.vector.tensor_add(out=v1[:ns, :], in0=v1[:ns, :],
                                         in1=q_i[:ns, :])

                mv1, rs1, nmr1 = ln_stats(v1, ns, nchunk)
                v2 = v_pool.tile([P, D], BF16, tag="v")
                apply_act(v2, v1, ns, rs1, nmr1, chunks=nchunk)

                if i == NT - 1:
                    # out[T-1] = tmp row 126 (SWDGE DMA casts bf16 -> fp32)
                    nc.gpsimd.dma_start(out=out_ap[T - 1:T, :],
                                        in_=v2[126:127, :])

                # v2 += shifted Q rows (prefetched into qs in the front)
                no2 = P if i < NT - 1 else P - 2
                nc.vector.tensor_add(out=v2[:no2, :], in0=v2[:no2, :],
                                     in1=qs[:no2, :])

                mv2, rs2, nmr2 = ln_stats(v2, no2, nchunk, want_nmr=True)
                out_t = o_pool.tile([P, D], FP32, tag="o")
                apply_act(out_t, v2, no2, rs2, nmr2, chunks=nchunk)
                nc.scalar.dma_start(out=out_ap[i * P + 1: i * P + 1 + no2, :],
                                    in_=out_t[:no2, :])

                if i == 0:
                    # out[0] = LN(x[0] + Q[1]): 1-row boundary
                    qrow = v_pool.tile([1, D], BF16, tag="qrow", bufs=1)
                    nc.sync.dma_start(out=qrow, in_=q_i[1:2, :])
                    v0 = v_pool.tile([1, D], BF16, tag="v0", bufs=1)
                    nc.sync.dma_start(out=v0[0:1, :], in_=xb_ap[0:1, :])
                    nc.vector.tensor_add(out=v0[0:1, :], in0=v0[0:1, :],
                                         in1=qrow[0:1, :])
                    mv0, rs0, _ = ln_stats(v0, 1, want_nmr=False)
                    o0 = o_pool.tile([1, D], FP32, tag="o0", bufs=1)
                    apply_dve(o0, v0, 1, rs0, mv0)
                    nc.sync.dma_start(out=out_ap[0:1, :], in_=o0[0:1, :])

            for i in reversed(range(NT)):
                emit_front(i)
                j = i + DELAY
                if j <= NT - 1:
                    emit_chain(j)
            for j in range(DELAY - 1, -1, -1):
                emit_chain(j)

    nc.compile()
    return nc


def _get_program(scale: float):
    key = round(float(scale), 9)
    if key not in _cache:
        _cache[key] = _build(float(scale))
    return _cache[key]


def _identity_ln_params(bp, gamma, beta):
    return (not np.any(bp)) and (not np.any(beta)) and np.all(gamma == 1.0)


def _reference_numpy(x, W1, b1, W2, b2, Wp, bp, gamma, beta):
    """Exact numpy port of the jax reference (emergency fallback only)."""
    import math

    def ln(v):
        mu = v.mean(-1, keepdims=True)
        var = ((v - mu) ** 2).mean(-1, keepdims=True)
        return (v - mu) / np.sqrt(var + EPS) * gamma + beta

    erf = np.vectorize(math.erf)
    x64 = x.astype(np.float32)
    pair = np.concatenate([x64[:, :-1], x64[:, 1:]], axis=-1)
    h0 = pair @ W1.T + b1
    h = 0.5 * h0 * (1.0 + erf(h0 / np.sqrt(2.0)))
    logits = h @ W2.T + b2
    e = np.exp(logits - logits.max(-1, keepdims=True))
    sm = e / e.sum(-1, keepdims=True)
    scale = sm.mean(-1, keepdims=True)
    Pm = x64 @ Wp.T + bp
    m = Pm[:, 1:] * scale
    mp = Pm[:, :-1] * scale
    tmp = np.concatenate([x64[:, :1], ln(x64[:, 1:] + mp)], axis=1)
    out = np.concatenate([ln(tmp[:, :-1] + m), tmp[:, -1:]], axis=1)
    return out.astype(np.float32)


def run_device(x, wT, scale, trace=False):
    """x: (B,T,D) fp32, wT: (D,D) fp32 (= Wp.T contiguous)."""
    nc = _get_program(scale)
    x8 = np.clip(x * SX, -240.0, 240.0).astype(F8NP)         # (B,T,D) fp8
    w8 = np.ascontiguousarray(
        np.clip(wT * SW, -240.0, 240.0).astype(F8NP)
        .reshape(NKP, 2, P, D).transpose(0, 2, 1, 3))        # (8,128,2,2048)
    in_maps = []
    for c in range(N_CORES):
        xb = np.ascontiguousarray(x[c].astype(BF16NP))
        # xTt[i, p, k, tt] = x8[i*128+tt, k*128+p]
        xTb = np.ascontiguousarray(
            x8[c].reshape(NT, P, NK, P).transpose(0, 3, 2, 1))
        in_maps.append({"xb": xb, "xTt": xTb, "w8": w8})
    res = run_bass_kernel_spmd(nc, in_maps, list(range(N_CORES)), trace=trace)
    out = np.stack([res.results[c]["out"] for c in range(N_CORES)], axis=0)
    return out, res


def kernel(x, W1, b1, W2, b2, Wp, bp, gamma, beta):
    x = np.asarray(x, dtype=np.float32)
    Wp = np.asarray(Wp, dtype=np.float32)
    bp = np.asarray(bp); gamma = np.asarray(gamma); beta = np.asarray(beta)
    b2 = np.asarray(b2)
    if x.shape != (B, T, D) or not _identity_ln_params(bp, gamma, beta):
        return _reference_numpy(np.asarray(x), np.asarray(W1), np.asarray(b1),
                                np.asarray(W2), b2, Wp, bp, gamma, beta)
    scale = 1.0 / float(b2.shape[0])
    wT = np.ascontiguousarray(Wp.T)
    out, _ = run_device(x, wT, scale, trace=False)
    return out
